# revision 4
# baseline (speedup 1.0000x reference)
"""GCN (2-layer GCNConv + ReLU) on 8 Trainium2 NeuronCores.

Strategy (dst-sharded message passing):
  - Nodes (and output rows) are sharded contiguously: core c owns rows
    [c*2500, (c+1)*2500). Edges (with self-loops appended) are partitioned by
    destination node and bucketed into 128-row destination blocks.
  - Layer 1 is computed aggregate-first ((A_hat X) W1 == A_hat (X W1)):
    the full input X is replicated to every core's HBM, so the layer-1
    gather of source rows is purely local (no collective needed).
  - Aggregation is expressed as small matmuls with a weighted one-hot
    selection matrix S^T[e, d] = norm_e * (dst_e == d), built on-device by the
    vector engine from per-edge (dst_local, norm) streams (one fused
    tensor_scalar op per 128-edge K-tile). Source rows are fetched with
    dma_gather (indexed row gather, HBM -> SBUF).
  - Layer 2 runs transform-first like the reference: h2 = x1 @ W2 sharded,
    one AllGather of h2 (20000 x 128) across the 8 cores, then local
    aggregation by destination + bias.
  - Normalization coefficients norm = dinv[src]*dinv[dst] are a pure function
    of edge_index and are computed on host while building the edge buckets.

The per-destination-block K-tile schedule (kt[b]) is shared by all 8 cores
(max over cores, padded with norm=0 dummy edges) so a single SPMD program
serves all cores with per-core input data.
"""

import os
import numpy as np

import concourse.bass as bass
import concourse.bacc as bacc
import concourse.mybir as mybir
import concourse.tile as tile
from concourse.bass_utils import run_bass_kernel_spmd

N_NODES = 20000
N_EDGES = 320000
D_IN = 256
D_HID = 256
D_OUT = 128
N_CORES = 8
ROWS = N_NODES // N_CORES          # 2500 rows per core
P = 128
NBLK = (ROWS + P - 1) // P         # 20 dst blocks per core (last has 68 rows)
LAST_ROWS = ROWS - (NBLK - 1) * P  # 68
NGRP = 5                           # dense-layer groups of 4 blocks (N=512)

USE_BF16 = bool(int(os.environ.get("GCN_BF16", "1")))
# Debug knobs: which phases to emit (subset of "ABCDE"), and a cap on the
# number of aggregation blocks traced in phases A/E. Both default to "full".
PHASES = os.environ.get("GCN_PHASES", "ABCDE")
BLK_CAP = int(os.environ.get("GCN_NBLKCAP", str(NBLK)))

# Set by test harnesses: when truthy, run_bass_kernel_spmd(trace=True) and the
# profile results are stashed in LAST_RESULTS.
TRACE = bool(int(os.environ.get("GCN_TRACE", "0")))
LAST_RESULTS = None

_CACHE = {}


def _prep(edge_index):
    """Bucket edges (with self-loops) by destination block; compute norms.

    Returns (kt, per_core) where kt is the shared K-tile schedule (length NBLK)
    and per_core[c] = dict(gidx, dloc, enorm) arrays for core c.
    """
    src = np.asarray(edge_index[0], dtype=np.int64)
    dst = np.asarray(edge_index[1], dtype=np.int64)
    loop = np.arange(N_NODES, dtype=np.int64)
    src_all = np.concatenate([src, loop])
    dst_all = np.concatenate([dst, loop])
    deg = np.bincount(dst_all, minlength=N_NODES).astype(np.float64)
    dinv = 1.0 / np.sqrt(deg)  # every node has a self-loop -> deg >= 1
    norm = (dinv[src_all] * dinv[dst_all]).astype(np.float32)

    order = np.argsort(dst_all, kind="stable")
    s_s = src_all[order]
    d_s = dst_all[order]
    n_s = norm[order]

    # Per (core, block) edge counts; edges sorted by dst so each bucket is a
    # contiguous slice.
    core_of = d_s // ROWS
    blk_of = (d_s % ROWS) // P
    bucket = core_of * NBLK + blk_of
    counts = np.bincount(bucket, minlength=N_CORES * NBLK).reshape(N_CORES, NBLK)
    kt = [int(np.ceil(counts[:, b].max() / P)) for b in range(NBLK)]
    kt_sum = sum(kt)

    starts = np.zeros(N_CORES * NBLK + 1, dtype=np.int64)
    np.cumsum(counts.reshape(-1), out=starts[1:])

    per_core = []
    for c in range(N_CORES):
        gidx = np.zeros((P, 8 * kt_sum), dtype=np.int16)
        dloc = np.zeros((P, kt_sum), dtype=np.float32)
        enorm = np.zeros((P, kt_sum), dtype=np.float32)
        koff = 0
        for b in range(NBLK):
            i0, i1 = starts[c * NBLK + b], starts[c * NBLK + b + 1]
            npad = kt[b] * P
            sp = np.zeros(npad, dtype=np.int64)
            dp = np.zeros(npad, dtype=np.float32)
            fp = np.zeros(npad, dtype=np.float32)
            cnt = i1 - i0
            sp[:cnt] = s_s[i0:i1]
            dp[:cnt] = (d_s[i0:i1] - (c * ROWS + b * P)).astype(np.float32)
            fp[:cnt] = n_s[i0:i1]
            # gather idx layout: element i at partition i%16, free i//16,
            # replicated across the 8 groups of 16 partitions.
            arr16 = sp.astype(np.int16).reshape(-1, 16).T  # [16, kt*8]
            gidx[:, 8 * koff : 8 * (koff + kt[b])] = np.tile(arr16, (8, 1))
            # S^T build layout: edge i = t*128 + p -> [p, koff+t]
            dloc[:, koff : koff + kt[b]] = dp.reshape(kt[b], P).T
            enorm[:, koff : koff + kt[b]] = fp.reshape(kt[b], P).T
            koff += kt[b]
        per_core.append({"gidx": gidx, "dloc": dloc, "enorm": enorm})
    return tuple(kt), per_core


def _build(kt):
    """Build the SPMD Bass program for the given shared K-tile schedule."""
    DT = mybir.dt.bfloat16 if USE_BF16 else mybir.dt.float32
    F32 = mybir.dt.float32
    kt_sum = sum(kt)

    nc = bacc.Bacc("TRN2", target_bir_lowering=False, debug=False,
                   num_devices=N_CORES)

    xin = nc.dram_tensor("xin", [N_NODES, D_IN], DT, kind="ExternalInput")
    w1 = nc.dram_tensor("w1", [D_IN, D_HID], DT, kind="ExternalInput")
    w2 = nc.dram_tensor("w2", [D_HID, D_OUT], DT, kind="ExternalInput")
    b1c = nc.dram_tensor("b1c", [D_HID, 1], F32, kind="ExternalInput")
    b2b = nc.dram_tensor("b2b", [P, D_OUT], F32, kind="ExternalInput")
    iotad = nc.dram_tensor("iotad", [P, P], F32, kind="ExternalInput")
    gidxd = nc.dram_tensor("gidxd", [P, 8 * kt_sum], mybir.dt.int16,
                           kind="ExternalInput")
    dlocd = nc.dram_tensor("dlocd", [P, kt_sum], F32, kind="ExternalInput")
    enormd = nc.dram_tensor("enormd", [P, kt_sum], F32, kind="ExternalInput")
    out = nc.dram_tensor("out", [ROWS, D_OUT], F32, kind="ExternalOutput")

    EQ = mybir.AluOpType.is_equal
    MUL = mybir.AluOpType.mult
    ADD = mybir.AluOpType.add
    RELU = mybir.ActivationFunctionType.Relu

    with tile.TileContext(nc) as tc:
        with (
            tc.tile_pool(name="const", bufs=1) as const,
            tc.tile_pool(name="resid", bufs=1) as resid,
            tc.tile_pool(name="eg", bufs=2) as epool,
            tc.tile_pool(name="st", bufs=6) as spool,
            tc.tile_pool(name="misc", bufs=3) as misc,
            tc.tile_pool(name="psA", bufs=4, space="PSUM") as psA,
            tc.tile_pool(name="psB", bufs=2, space="PSUM") as psB,
            tc.tile_pool(name="dram", bufs=1, space="DRAM") as dram,
        ):
            # ---- constants / graph metadata ----
            iota_f = const.tile([P, P], F32)
            nc.sync.dma_start(iota_f[:], iotad[:])
            gidx = const.tile([P, 8 * kt_sum], mybir.dt.int16)
            nc.sync.dma_start(gidx[:], gidxd[:])
            dloc = const.tile([P, kt_sum], F32)
            nc.sync.dma_start(dloc[:], dlocd[:])
            enorm = const.tile([P, kt_sum], F32)
            nc.sync.dma_start(enorm[:], enormd[:])
            w1s = []
            for k in range(2):
                w1k = const.tile([P, D_HID], DT, name=f"w1s{k}")
                nc.sync.dma_start(w1k[:], w1[k * P : (k + 1) * P, :])
                w1s.append(w1k)
            w2s = []
            for k in range(2):
                w2k = const.tile([P, D_OUT], DT, name=f"w2s{k}")
                nc.sync.dma_start(w2k[:], w2[k * P : (k + 1) * P, :])
                w2s.append(w2k)
            b1s = []
            for k in range(2):
                b1k = const.tile([P, 1], F32, name=f"b1s{k}")
                nc.sync.dma_start(b1k[:], b1c[k * P : (k + 1) * P, :])
                b1s.append(b1k)
            b2t = const.tile([P, D_OUT], F32)
            nc.sync.dma_start(b2t[:], b2b[:])

            # resident activations (feature-major, dst columns padded to 2560)
            aggXT = [resid.tile([P, NBLK * P], DT, name=f"aggXT{h}")
                     for h in range(2)]
            x1T = [resid.tile([P, NBLK * P], DT, name=f"x1T{h}")
                   for h in range(2)]

            h2s = dram.tile([ROWS, D_OUT], DT)
            h2f = dram.tile([N_NODES, D_OUT], DT, addr_space="Shared")

            # ---- phase A: layer-1 aggregation aggX^T = E^T @ S^T ----
            koff = 0
            for b in range(BLK_CAP if "A" in PHASES else 0):
                ktb = kt[b]
                et = epool.tile([P, ktb, D_IN], DT, tag="eg1", name=f"et{b}")
                nc.gpsimd.dma_gather(
                    et[:], xin[:], gidx[:, 8 * koff : 8 * (koff + ktb)],
                    num_idxs=ktb * P, num_idxs_reg=ktb * P, elem_size=D_IN,
                    single_packet=False,
                )
                p0 = psA.tile([P, P], F32, tag="pA", name=f"pa0_{b}")
                p1 = psA.tile([P, P], F32, tag="pA", name=f"pa1_{b}")
                for t in range(ktb):
                    st = spool.tile([P, P], DT, tag="st", name=f"sA{b}_{t}")
                    nc.vector.tensor_scalar(
                        st[:], iota_f[:],
                        dloc[:, koff + t : koff + t + 1],
                        enorm[:, koff + t : koff + t + 1],
                        EQ, MUL,
                    )
                    nc.tensor.matmul(p0[:], lhsT=et[:, t, 0:P], rhs=st[:],
                                     start=(t == 0), stop=(t == ktb - 1))
                    nc.tensor.matmul(p1[:], lhsT=et[:, t, P:D_IN], rhs=st[:],
                                     start=(t == 0), stop=(t == ktb - 1))
                nc.vector.tensor_copy(aggXT[0][:, b * P : (b + 1) * P], p0[:])
                nc.vector.tensor_copy(aggXT[1][:, b * P : (b + 1) * P], p1[:])
                koff += ktb

            # ---- phase B: x1^T = relu(W1^T aggX^T + b1) ----
            for g in range(NGRP if "B" in PHASES else 0):
                cs = slice(g * 4 * P, (g + 1) * 4 * P)
                for m in range(2):
                    px = psB.tile([P, 4 * P], F32, tag="pB", name=f"px{g}_{m}")
                    for k in range(2):
                        nc.tensor.matmul(px[:],
                                         lhsT=w1s[k][:, m * P : (m + 1) * P],
                                         rhs=aggXT[k][:, cs],
                                         start=(k == 0), stop=(k == 1))
                    nc.scalar.activation(x1T[m][:, cs], px[:], RELU,
                                         bias=b1s[m][:, 0:1])

            # ---- phase C: h2 = x1 @ W2 (node-major), DMA to h2 shard ----
            for tmt in range(NBLK if "C" in PHASES else 0):
                mt = P if tmt < NBLK - 1 else LAST_ROWS
                ph = psB.tile([P, D_OUT], F32, tag="pB", name=f"ph{tmt}")
                for k in range(2):
                    nc.tensor.matmul(ph[0:mt, :],
                                     lhsT=x1T[k][:, tmt * P : tmt * P + mt],
                                     rhs=w2s[k][:],
                                     start=(k == 0), stop=(k == 1))
                h2t = misc.tile([P, D_OUT], DT, tag="h2t", name=f"h2t{tmt}")
                nc.vector.tensor_copy(h2t[0:mt, :], ph[0:mt, :])
                nc.sync.dma_start(h2s[tmt * P : tmt * P + mt, :], h2t[0:mt, :])

            # ---- phase D: AllGather h2 across the 8 cores ----
            if "D" in PHASES:
                nc.gpsimd.collective_compute(
                "AllGather", mybir.AluOpType.bypass,
                    replica_groups=[list(range(N_CORES))],
                    ins=[h2s.opt()], outs=[h2f.opt()],
                )

            # ---- phase E: layer-2 aggregation out = S @ E2 + b2 ----
            koff = 0
            for b in range(BLK_CAP if "E" in PHASES else 0):
                ktb = kt[b]
                mt = P if b < NBLK - 1 else LAST_ROWS
                e2 = epool.tile([P, ktb, D_OUT], DT, tag="eg2", name=f"e2_{b}")
                nc.gpsimd.dma_gather(
                    e2[:], h2f[:], gidx[:, 8 * koff : 8 * (koff + ktb)],
                    num_idxs=ktb * P, num_idxs_reg=ktb * P, elem_size=D_OUT,
                    single_packet=False,
                )
                po = psA.tile([P, D_OUT], F32, tag="pA", name=f"po{b}")
                for t in range(ktb):
                    st = spool.tile([P, P], DT, tag="st", name=f"sE{b}_{t}")
                    nc.vector.tensor_scalar(
                        st[:], iota_f[:],
                        dloc[:, koff + t : koff + t + 1],
                        enorm[:, koff + t : koff + t + 1],
                        EQ, MUL,
                    )
                    nc.tensor.matmul(po[:], lhsT=st[:], rhs=e2[:, t, :],
                                     start=(t == 0), stop=(t == ktb - 1))
                ot = misc.tile([P, D_OUT], F32, tag="ot", name=f"ot{b}")
                nc.vector.tensor_tensor(ot[0:mt, :], po[0:mt, :], b2t[0:mt, :],
                                        op=ADD)
                nc.sync.dma_start(out[b * P : b * P + mt, :], ot[0:mt, :])
                koff += ktb

    nc.compile()
    return nc


def kernel(view, edge_index, W1, b1, W2, b2):
    global LAST_RESULTS
    view = np.ascontiguousarray(np.asarray(view, dtype=np.float32))
    W1 = np.asarray(W1, dtype=np.float32)
    b1 = np.asarray(b1, dtype=np.float32)
    W2 = np.asarray(W2, dtype=np.float32)
    b2 = np.asarray(b2, dtype=np.float32)

    kt, per_core = _prep(edge_index)
    if kt not in _CACHE:
        _CACHE[kt] = _build(kt)
    nc = _CACHE[kt]

    np_dt = np.dtype(mybir.dt.np(mybir.dt.bfloat16)) if USE_BF16 else np.float32
    xin = np.ascontiguousarray(view.astype(np_dt))
    w1 = np.ascontiguousarray(W1.astype(np_dt))
    w2 = np.ascontiguousarray(W2.astype(np_dt))
    b1c = np.ascontiguousarray(b1.reshape(D_HID, 1))
    b2b = np.ascontiguousarray(np.broadcast_to(b2[None, :], (P, D_OUT)))
    iotad = np.ascontiguousarray(
        np.broadcast_to(np.arange(P, dtype=np.float32)[None, :], (P, P)))

    in_maps = []
    for c in range(N_CORES):
        m = {"xin": xin, "w1": w1, "w2": w2, "b1c": b1c, "b2b": b2b,
             "iotad": iotad, "gidxd": per_core[c]["gidx"],
             "dlocd": per_core[c]["dloc"], "enormd": per_core[c]["enorm"]}
        in_maps.append(m)

    res = run_bass_kernel_spmd(nc, in_maps, list(range(N_CORES)), trace=TRACE)
    LAST_RESULTS = res
    return np.concatenate([res.results[c]["out"] for c in range(N_CORES)], axis=0)
